# revision 66
# baseline (speedup 1.0000x reference)
"""MoE gate (router) kernel for Trainium2, SPMD across 8 NeuronCores.

Math (per row): logits = x @ W.T + bias; probs = softmax(logits);
top-8 (vals desc, idx), dense_gate = scatter(topk_vals).

Layout: x is sharded row-wise across 8 cores (2048 rows each) and
host-transposed so the contraction dim (2048) lies on SBUF partitions.
Per core the rows are processed in 3 chase-blocks (1024/512/512 rows)
so early blocks' softmax/top-k epilogues overlap later blocks' x DMAs.
Logits^T accumulate in PSUM over 16 float32r k-chunk matmuls, bias is
added on the PSUM->SBUF move, PE-transpose yields (128 rows x 64
experts) tiles, then ACT exp (max-subtracted, row-sum accumulated) and
DVE hardware top-8 (max / max_index / match_replace) produce all three
outputs, staged in SBUF and stored with one DMA per 512-row block per
output tensor.
"""

import sys
from contextlib import ExitStack

import numpy as np

try:
    import concourse.bass as bass
except ImportError:  # fresh grading dir without the default PYTHONPATH
    for p in ("/root/.axon_site", "/root/.axon_site/_ro/trn_rl_repo",
              "/root/.axon_site/_ro/pypackages", "/opt/trn_rl_repo"):
        if p not in sys.path:
            sys.path.append(p)
    import concourse.bass as bass

import concourse.tile as tile
from concourse import bacc, mybir
from concourse.bass_utils import run_bass_kernel_spmd
from concourse.masks import make_identity

N_TOTAL = 16384
D = 2048
E = 64
TOPK = 8
N_CORES = 8
R = N_TOTAL // N_CORES   # 2048 rows per core
P = 128                  # SBUF partitions
KC = D // P              # 16 contraction chunks
RBS = 512                # rows per epilogue block / matmul free dim
TPB = RBS // P           # 4 row tiles per 512-block
# (row0, nrows, k-chunk DMA groups): each group is one input DMA.
# Chase blocks shrink toward the end so the post-wire tail (final block's
# matmuls + softmax/top-k epilogue) is as short as possible; the very last
# DMA carries only 2 k-chunks.
BLOCKS = [
    (0, 1024, [[i] for i in range(16)]),
    (1024, 512, [[2 * i, 2 * i + 1] for i in range(8)]),
    (1536, 128, [list(range(8)), list(range(8, 16))]),
    (1664, 128, [list(range(8)), list(range(8, 16))]),
    (1792, 128, [list(range(8)), list(range(8, 16))]),
    (1920, 128, [list(range(6)), [6, 7], [8, 9], [10, 11],
                 [12, 13], [14], [15]]),
]
OW = E + 2 * TOPK          # 80 used output cols: gate | vals | idx-as-float
OWP = 128                  # padded to 512B rows so DMA lines avoid the
                           # small-element throughput penalty

F32 = mybir.dt.float32
F32R = mybir.dt.float32r
I32 = mybir.dt.int32
U32 = mybir.dt.uint32

_prog = None


def _build():
    nc = bacc.Bacc("TRN2", target_bir_lowering=False, debug=False,
                   enable_asserts=True, num_devices=N_CORES)
    xt_d = nc.declare_dram_parameter("xt", [D, R], F32, isOutput=False)
    wt_d = nc.declare_dram_parameter("wt", [P, KC * E], F32, isOutput=False)
    b_d = nc.declare_dram_parameter("bias", [E, 1], F32, isOutput=False)
    br_d = nc.declare_dram_parameter("biasr", [1, E], F32, isOutput=False)
    out_d = nc.declare_dram_parameter("out", [R, OWP], F32, isOutput=True)

    AF = mybir.ActivationFunctionType
    with tile.TileContext(nc) as tc, ExitStack() as ctx:
        const = ctx.enter_context(tc.tile_pool(name="const", bufs=1))
        xp = ctx.enter_context(tc.tile_pool(name="xp", bufs=12))
        pending_out = []

        def load_tile(row0, nrows, chunks):
            n = len(chunks)
            c0 = chunks[0]
            xt = xp.tile([P, n * nrows], F32)
            src = xt_d[c0 * P:(c0 + n) * P, row0:row0 + nrows]
            nc.sync.dma_start(
                xt[:], src.rearrange("(c p) n -> p c n", c=n, p=P))
            return xt

        # first x DMA rides Pool SWDGE: Pool's sequencer config is ~25ns vs
        # SP's 565ns, so the wire starts ~200ns earlier.
        c0 = BLOCKS[0][2][0][0]
        first_xt = xp.tile([P, 1024], F32)
        nc.gpsimd.dma_start(
            first_xt[:],
            xt_d[c0 * P:(c0 + 1) * P, 0:1024].rearrange(
                "(c p) n -> p c n", c=1, p=P))
        psA = ctx.enter_context(
            tc.tile_pool(name="psA", bufs=1, space=bass.MemorySpace.PSUM))
        sbT = ctx.enter_context(tc.tile_pool(name="sbT", bufs=2))
        psB = ctx.enter_context(
            tc.tile_pool(name="psB", bufs=2, space=bass.MemorySpace.PSUM))
        psC = ctx.enter_context(
            tc.tile_pool(name="psC", bufs=2, space=bass.MemorySpace.PSUM))
        wk = ctx.enter_context(tc.tile_pool(name="wk", bufs=2))
        ob = ctx.enter_context(tc.tile_pool(name="ob", bufs=8))
        wt = const.tile([P, KC * E], F32)
        nc.sync.dma_start(wt[:], wt_d[:])
        bias = const.tile([E, 1], F32)
        nc.sync.dma_start(bias[:], b_d[:])
        biasr = const.tile([1, E], F32)
        nc.sync.dma_start(biasr[:], br_d[:])
        ident = const.tile([E, E], F32)
        make_identity(nc, ident[:])
        ones = const.tile([1, P], F32)
        nc.gpsimd.memset(ones[:], 1.0)

        def epilogue(row0, width, psT):
            nsub = width // P
            lt = sbT.tile([E, RBS], F32)
            lt = lt[:, :width]
            nc.scalar.activation(lt, psT[:], AF.Identity, bias=bias[:])
            comb = ob.tile([P, TPB * OWP], F32)
            for t in range(nsub):
                psR = psB.tile([P, E], F32)
                nc.tensor.transpose(psR[:], lt[:, t * P:(t + 1) * P],
                                    ident[:])
                negmax = wk.tile([P, 1], F32)
                nc.vector.tensor_reduce(negmax[:], psR[:],
                                        mybir.AxisListType.X,
                                        mybir.AluOpType.max, negate=True)
                exps = wk.tile([P, E], F32)
                rsum = wk.tile([P, 1], F32)
                nc.scalar.activation(exps[:], psR[:], AF.Exp,
                                     bias=negmax[:], accum_out=rsum[:])
                rcp = wk.tile([P, 1], F32)
                nc.vector.reciprocal(rcp[:], rsum[:])
                probs = wk.tile([P, E], F32)
                nc.vector.tensor_scalar(probs[:], exps[:], rcp[:], None,
                                        mybir.AluOpType.mult)
                base = t * OWP
                v8 = comb[:, base + E:base + E + TOPK]
                nc.vector.max(v8, probs[:])
                idxu = wk.tile([P, TOPK], U32)
                nc.vector.max_index(idxu[:], v8, probs[:])
                nc.vector.tensor_copy(
                    comb[:, base + E + TOPK:base + OW], idxu[:])
                masked = wk.tile([P, E], F32)
                nc.vector.match_replace(masked[:], v8, probs[:], 0.0)
                nc.vector.tensor_tensor(comb[:, base:base + E],
                                        probs[:], masked[:],
                                        mybir.AluOpType.subtract)
            sl = slice(row0, row0 + width)
            # one combined store per block: gate|vals|idx ride in a single
            # DMA. Deferred: the wire is FIFO, so output DMAs must enter it
            # only after the final input DMA or they delay the tail.
            pending_out.append(
                (out_d[sl, :].rearrange("(t p) c -> p t c", t=nsub, p=P),
                 comb[:, :nsub * OWP]))

        def tail_epilogue(row0, psR, defer):
            # last block keeps rows on partitions (no transpose, no
            # PSUM->SBUF move): DVE/ACT read PSUM directly, bias came in
            # through the ones x biasr matmul.
            negmax = wk.tile([P, 1], F32)
            nc.vector.tensor_reduce(negmax[:], psR[:], mybir.AxisListType.X,
                                    mybir.AluOpType.max, negate=True)
            exps = wk.tile([P, E], F32)
            rsum = wk.tile([P, 1], F32)
            nc.scalar.activation(exps[:], psR[:], AF.Exp,
                                 bias=negmax[:], accum_out=rsum[:])
            rcp = wk.tile([P, 1], F32)
            nc.vector.reciprocal(rcp[:], rsum[:])
            probs = wk.tile([P, E], F32)
            nc.vector.tensor_scalar(probs[:], exps[:], rcp[:], None,
                                    mybir.AluOpType.mult)
            comb = ob.tile([P, OWP], F32)
            v8 = comb[:, E:E + TOPK]
            nc.vector.max(v8, probs[:])
            idxu = wk.tile([P, TOPK], U32)
            nc.vector.max_index(idxu[:], v8, probs[:])
            nc.vector.tensor_copy(comb[:, E + TOPK:OW], idxu[:])
            masked = wk.tile([P, E], F32)
            nc.vector.match_replace(masked[:], v8, probs[:], 0.0)
            nc.vector.tensor_tensor(comb[:, :E], probs[:], masked[:],
                                    mybir.AluOpType.subtract)
            if defer:
                pending_out.append((out_d[row0:row0 + P, :], comb[:]))
            else:
                nc.sync.dma_start(out_d[row0:row0 + P, :], comb[:])

        for bi, (row0, nrows, groups) in enumerate(BLOCKS):
            last = bi == len(BLOCKS) - 1
            if nrows == P:
                psR = psC.tile([P, E], F32, name="psR")
                nc.tensor.matmul(psR[:], ones[:], biasr[:],
                                 start=True, stop=False)
                for chunks in groups:
                    xt = load_tile(row0, nrows, chunks)
                    for ci, kc in enumerate(chunks):
                        nc.tensor.matmul(
                            psR[:], xt[:, ci * nrows:(ci + 1) * nrows],
                            wt[:, kc * E:(kc + 1) * E],
                            start=False, stop=(kc == KC - 1))
                if last:
                    # all inputs are now issued: release the staged output
                    # DMAs (SP HWDGE, so they gen in order after the last
                    # input gen and enter the wire FIFO behind it)
                    for dst, src in pending_out:
                        nc.sync.dma_start(dst, src)
                tail_epilogue(row0, psR, defer=not last)
                continue
            w = min(nrows, RBS)
            nj = max(nrows // RBS, 1)
            psTs = [psA.tile([E, w], F32, name=f"psT_{row0}_{j}")
                    for j in range(nj)]
            for gi, chunks in enumerate(groups):
                xt = (first_xt if bi == 0 and gi == 0
                      else load_tile(row0, nrows, chunks))
                for ci, kc in enumerate(chunks):
                    if nj > 1:
                        for j in range(nj):
                            nc.tensor.matmul(
                                psTs[j][:], wt[:, kc * E:(kc + 1) * E],
                                xt[:, j * RBS:(j + 1) * RBS],
                                start=(kc == 0), stop=(kc == KC - 1))
                    else:
                        nc.tensor.matmul(
                            psTs[0][:], wt[:, kc * E:(kc + 1) * E],
                            xt[:, ci * nrows:(ci + 1) * nrows],
                            start=(kc == 0), stop=(kc == KC - 1))
            for j in range(nj):
                epilogue(row0 + j * RBS, w, psTs[j])

    nc.compile()
    return nc


def _prepare_in_maps(x, weight, bias):
    x = np.ascontiguousarray(np.asarray(x, dtype=np.float32))
    weight = np.ascontiguousarray(np.asarray(weight, dtype=np.float32))
    bias = np.asarray(bias, dtype=np.float32).reshape(E, 1).copy()
    biasr = np.ascontiguousarray(bias.reshape(1, E))
    # wt[p, kc*E + e] = weight[e, kc*P + p]
    wt = np.ascontiguousarray(
        weight.T.reshape(KC, P, E).transpose(1, 0, 2).reshape(P, KC * E))
    in_maps = []
    for c in range(N_CORES):
        xtc = np.ascontiguousarray(x[c * R:(c + 1) * R].T)  # (D, R)
        in_maps.append({"xt": xtc, "wt": wt, "bias": bias, "biasr": biasr})
    return in_maps


def _gather(results):
    comb = np.concatenate([results[c]["out"] for c in range(N_CORES)], axis=0)
    gate = np.ascontiguousarray(comb[:, :E], dtype=np.float32)
    vals = np.ascontiguousarray(comb[:, E:E + TOPK], dtype=np.float32)
    idx = comb[:, E + TOPK:OW].astype(np.int32)
    return vals, idx, gate


def _run(in_maps, **kw):
    global _prog
    if _prog is None:
        _prog = _build()
    return run_bass_kernel_spmd(_prog, in_maps, list(range(N_CORES)), **kw)


def kernel(x, weight, bias):
    in_maps = _prepare_in_maps(x, weight, bias)
    return _gather(_run(in_maps).results)


# revision 72
# speedup vs baseline: 1.2791x; 1.2791x over previous
"""MoE gate (router) kernel for Trainium2, SPMD across 8 NeuronCores.

Math (per row): logits = x @ W.T + bias; probs = softmax(logits);
top-8 (vals desc, idx), dense_gate = scatter(topk_vals).

Layout: x is sharded row-wise across 8 cores (2048 rows each) and
host-transposed so the contraction dim (2048) lies on SBUF partitions.
Per core the rows are processed in 3 chase-blocks (1024/512/512 rows)
so early blocks' softmax/top-k epilogues overlap later blocks' x DMAs.
Logits^T accumulate in PSUM over 16 float32r k-chunk matmuls, bias is
added on the PSUM->SBUF move, PE-transpose yields (128 rows x 64
experts) tiles, then ACT exp (max-subtracted, row-sum accumulated) and
DVE hardware top-8 (max / max_index / match_replace) produce all three
outputs, staged in SBUF and stored with one DMA per 512-row block per
output tensor.
"""

import sys
from contextlib import ExitStack

import numpy as np

try:
    import concourse.bass as bass
except ImportError:  # fresh grading dir without the default PYTHONPATH
    for p in ("/root/.axon_site", "/root/.axon_site/_ro/trn_rl_repo",
              "/root/.axon_site/_ro/pypackages", "/opt/trn_rl_repo"):
        if p not in sys.path:
            sys.path.append(p)
    import concourse.bass as bass

import concourse.tile as tile
from concourse import bacc, mybir
from concourse.bass_utils import run_bass_kernel_spmd
from concourse.masks import make_identity

N_TOTAL = 16384
D = 2048
E = 64
TOPK = 8
N_CORES = 8
R = N_TOTAL // N_CORES   # 2048 rows per core
P = 128                  # SBUF partitions
KC = D // P              # 16 contraction chunks
RBS = 512                # rows per epilogue block / matmul free dim
TPB = RBS // P           # 4 row tiles per 512-block
# (row0, nrows, k-chunk DMA groups): each group is one input DMA.
# Chase blocks shrink toward the end so the post-wire tail (final block's
# matmuls + softmax/top-k epilogue) is as short as possible; the very last
# DMA carries only 2 k-chunks.
BLOCKS = [
    (0, 1024, [[i] for i in range(16)]),
    (1024, 512, [[2 * i, 2 * i + 1] for i in range(8)]),
    (1536, 128, [list(range(8)), list(range(8, 16))]),
    (1664, 128, [list(range(8)), list(range(8, 16))]),
    (1792, 128, [list(range(8)), list(range(8, 16))]),
    (1920, 128, [list(range(6)), [6, 7], [8, 9], [10, 11],
                 [12, 13], [14], [15]]),
]
OW = E + 2 * TOPK          # 80 used output cols: gate | vals | idx-as-float
OWP = 128                  # padded to 512B rows so DMA lines avoid the
                           # small-element throughput penalty

F32 = mybir.dt.float32
FP16 = mybir.dt.float16
I32 = mybir.dt.int32
U32 = mybir.dt.uint32


def _dma_offsets():
    # x rides as fp16 hi/lo pairs packed on the host in exactly the SBUF
    # tile layout each DMA wants, so every transfer is one contiguous
    # [128, L] slice (large lines, no scatter)
    offs, off = [], 0
    for row0, nrows, groups in BLOCKS:
        g = []
        for chunks in groups:
            g.append(off)
            off += 2 * len(chunks) * nrows
        offs.append(g)
    return offs, off


OFFS, F_TOT = _dma_offsets()

_prog = None


def _build():
    nc = bacc.Bacc("TRN2", target_bir_lowering=False, debug=False,
                   enable_asserts=True, num_devices=N_CORES)
    xt_d = nc.declare_dram_parameter("xt", [P, F_TOT], FP16, isOutput=False)
    wt_d = nc.declare_dram_parameter("wt", [P, 2 * KC * E], FP16,
                                     isOutput=False)
    b_d = nc.declare_dram_parameter("bias", [E, 1], F32, isOutput=False)
    br_d = nc.declare_dram_parameter("biasr", [1, E], F32, isOutput=False)
    out_d = nc.declare_dram_parameter("out", [R, OWP], F32, isOutput=True)

    AF = mybir.ActivationFunctionType
    with tile.TileContext(nc) as tc, ExitStack() as ctx:
        const = ctx.enter_context(tc.tile_pool(name="const", bufs=1))
        xp = ctx.enter_context(tc.tile_pool(name="xp", bufs=12))
        pending_out = []

        def load_tile(bi, gi, nrows, nch):
            xt = xp.tile([P, 2 * nch * nrows], FP16)
            off = OFFS[bi][gi]
            nc.sync.dma_start(xt[:], xt_d[:, off:off + 2 * nch * nrows])
            return xt

        # first x DMA rides Pool SWDGE: Pool's sequencer config is ~25ns vs
        # SP's 565ns, so the wire starts ~200ns earlier.
        first_xt = xp.tile([P, 2 * 1024], FP16)
        nc.gpsimd.dma_start(first_xt[:], xt_d[:, 0:2 * 1024])
        psA = ctx.enter_context(
            tc.tile_pool(name="psA", bufs=1, space=bass.MemorySpace.PSUM))
        sbT = ctx.enter_context(tc.tile_pool(name="sbT", bufs=2))
        psB = ctx.enter_context(
            tc.tile_pool(name="psB", bufs=2, space=bass.MemorySpace.PSUM))
        psC = ctx.enter_context(
            tc.tile_pool(name="psC", bufs=2, space=bass.MemorySpace.PSUM))
        wk = ctx.enter_context(tc.tile_pool(name="wk", bufs=2))
        ob = ctx.enter_context(tc.tile_pool(name="ob", bufs=8))
        wt = const.tile([P, 2 * KC * E], FP16)
        nc.sync.dma_start(wt[:], wt_d[:])
        bias = const.tile([E, 1], F32)
        nc.sync.dma_start(bias[:], b_d[:])
        biasr = const.tile([1, E], F32)
        nc.sync.dma_start(biasr[:], br_d[:])
        ident = const.tile([E, E], F32)
        make_identity(nc, ident[:])
        ones = const.tile([1, P], F32)
        nc.gpsimd.memset(ones[:], 1.0)

        def epilogue(row0, width, psT):
            nsub = width // P
            lt = sbT.tile([E, RBS], F32)
            lt = lt[:, :width]
            nc.scalar.activation(lt, psT[:], AF.Identity, bias=bias[:])
            comb = ob.tile([P, TPB * OWP], F32)
            for t in range(nsub):
                psR = psB.tile([P, E], F32)
                nc.tensor.transpose(psR[:], lt[:, t * P:(t + 1) * P],
                                    ident[:])
                negmax = wk.tile([P, 1], F32)
                nc.vector.tensor_reduce(negmax[:], psR[:],
                                        mybir.AxisListType.X,
                                        mybir.AluOpType.max, negate=True)
                exps = wk.tile([P, E], F32)
                rsum = wk.tile([P, 1], F32)
                nc.scalar.activation(exps[:], psR[:], AF.Exp,
                                     bias=negmax[:], accum_out=rsum[:])
                rcp = wk.tile([P, 1], F32)
                nc.vector.reciprocal(rcp[:], rsum[:])
                probs = wk.tile([P, E], F32)
                nc.vector.tensor_scalar(probs[:], exps[:], rcp[:], None,
                                        mybir.AluOpType.mult)
                base = t * OWP
                v8 = comb[:, base + E:base + E + TOPK]
                nc.vector.max(v8, probs[:])
                idxu = wk.tile([P, TOPK], U32)
                nc.vector.max_index(idxu[:], v8, probs[:])
                nc.vector.tensor_copy(
                    comb[:, base + E + TOPK:base + OW], idxu[:])
                masked = wk.tile([P, E], F32)
                nc.vector.match_replace(masked[:], v8, probs[:], 0.0)
                nc.vector.tensor_tensor(comb[:, base:base + E],
                                        probs[:], masked[:],
                                        mybir.AluOpType.subtract)
            sl = slice(row0, row0 + width)
            # one combined store per block: gate|vals|idx ride in a single
            # DMA. Deferred: the wire is FIFO, so output DMAs must enter it
            # only after the final input DMA or they delay the tail.
            pending_out.append(
                (out_d[sl, :].rearrange("(t p) c -> p t c", t=nsub, p=P),
                 comb[:, :nsub * OWP]))

        def tail_epilogue(row0, psR, defer):
            # last block keeps rows on partitions (no transpose, no
            # PSUM->SBUF move): DVE/ACT read PSUM directly, bias came in
            # through the ones x biasr matmul.
            negmax = wk.tile([P, 1], F32)
            nc.vector.tensor_reduce(negmax[:], psR[:], mybir.AxisListType.X,
                                    mybir.AluOpType.max, negate=True)
            exps = wk.tile([P, E], F32)
            rsum = wk.tile([P, 1], F32)
            nc.scalar.activation(exps[:], psR[:], AF.Exp,
                                 bias=negmax[:], accum_out=rsum[:])
            rcp = wk.tile([P, 1], F32)
            nc.vector.reciprocal(rcp[:], rsum[:])
            probs = wk.tile([P, E], F32)
            nc.vector.tensor_scalar(probs[:], exps[:], rcp[:], None,
                                    mybir.AluOpType.mult)
            comb = ob.tile([P, OWP], F32)
            v8 = comb[:, E:E + TOPK]
            nc.vector.max(v8, probs[:])
            idxu = wk.tile([P, TOPK], U32)
            nc.vector.max_index(idxu[:], v8, probs[:])
            nc.vector.tensor_copy(comb[:, E + TOPK:OW], idxu[:])
            masked = wk.tile([P, E], F32)
            nc.vector.match_replace(masked[:], v8, probs[:], 0.0)
            nc.vector.tensor_tensor(comb[:, :E], probs[:], masked[:],
                                    mybir.AluOpType.subtract)
            if defer:
                pending_out.append((out_d[row0:row0 + P, :], comb[:]))
            else:
                nc.sync.dma_start(out_d[row0:row0 + P, :], comb[:])

        # fp16 hi/lo 3-term matmul: x@W = xh@Wh + xh@Wl + xl@Wh (+ O(2^-22)).
        # 3 one-pass fp16 matmuls beat one 4-pass fp32 matmul on PE while
        # keeping logits at fp32-grade accuracy (verified: 0 idx flips).
        def wsec(kc, h):
            s = 2 * kc + h
            return wt[:, s * E:(s + 1) * E]

        for bi, (row0, nrows, groups) in enumerate(BLOCKS):
            last = bi == len(BLOCKS) - 1
            if nrows == P:
                psR = psC.tile([P, E], F32, name="psR")
                nc.tensor.matmul(psR[:], ones[:], biasr[:],
                                 start=True, stop=False)
                for gi, chunks in enumerate(groups):
                    xt = load_tile(bi, gi, nrows, len(chunks))
                    for ci, kc in enumerate(chunks):
                        xh = xt[:, (2 * ci) * nrows:(2 * ci + 1) * nrows]
                        xl = xt[:, (2 * ci + 1) * nrows:(2 * ci + 2) * nrows]
                        fin = kc == KC - 1
                        nc.tensor.matmul(psR[:], xh, wsec(kc, 0),
                                         start=False, stop=False)
                        nc.tensor.matmul(psR[:], xh, wsec(kc, 1),
                                         start=False, stop=False)
                        nc.tensor.matmul(psR[:], xl, wsec(kc, 0),
                                         start=False, stop=fin)
                if last:
                    # all inputs are now issued: release the staged output
                    # DMAs (SP HWDGE, so they gen in order after the last
                    # input gen and enter the wire FIFO behind it)
                    for dst, src in pending_out:
                        nc.sync.dma_start(dst, src)
                tail_epilogue(row0, psR, defer=not last)
                continue
            w = min(nrows, RBS)
            nj = max(nrows // RBS, 1)
            psTs = [psA.tile([E, w], F32, name=f"psT_{row0}_{j}")
                    for j in range(nj)]
            for gi, chunks in enumerate(groups):
                xt = (first_xt if bi == 0 and gi == 0
                      else load_tile(bi, gi, nrows, len(chunks)))
                for ci, kc in enumerate(chunks):
                    hb = (2 * ci) * nrows
                    lb = (2 * ci + 1) * nrows
                    for j in range(nj):
                        xh = xt[:, hb + j * RBS:hb + j * RBS + w]
                        xl = xt[:, lb + j * RBS:lb + j * RBS + w]
                        st = kc == 0
                        fin = kc == KC - 1
                        nc.tensor.matmul(psTs[j][:], wsec(kc, 0), xh,
                                         start=st, stop=False)
                        nc.tensor.matmul(psTs[j][:], wsec(kc, 1), xh,
                                         start=False, stop=False)
                        nc.tensor.matmul(psTs[j][:], wsec(kc, 0), xl,
                                         start=False, stop=fin)
            for j in range(nj):
                epilogue(row0 + j * RBS, w, psTs[j])

    nc.compile()
    return nc


def _prepare_in_maps(x, weight, bias):
    x = np.ascontiguousarray(np.asarray(x, dtype=np.float32))
    weight = np.ascontiguousarray(np.asarray(weight, dtype=np.float32))
    bias = np.asarray(bias, dtype=np.float32).reshape(E, 1).copy()
    biasr = np.ascontiguousarray(bias.reshape(1, E))
    wh = weight.astype(np.float16)
    wl = (weight - wh.astype(np.float32)).astype(np.float16)
    # wt[p, (2*kc+h)*E + e] = w_part[e, kc*P + p]
    stack = np.stack([wh.T.reshape(KC, P, E), wl.T.reshape(KC, P, E)],
                     axis=1)  # [KC, 2, P, E]
    wt = np.ascontiguousarray(
        stack.transpose(2, 0, 1, 3).reshape(P, 2 * KC * E))
    in_maps = []
    for c in range(N_CORES):
        xc = x[c * R:(c + 1) * R]  # (R, D)
        xh = xc.astype(np.float16)
        xl = (xc - xh.astype(np.float32)).astype(np.float16)
        xhT = np.ascontiguousarray(xh.T)  # (D, R)
        xlT = np.ascontiguousarray(xl.T)
        buf = np.empty((P, F_TOT), np.float16)
        for (row0, nrows, groups), goffs in zip(BLOCKS, OFFS):
            for chunks, off in zip(groups, goffs):
                for ci, ch in enumerate(chunks):
                    b0 = off + 2 * ci * nrows
                    buf[:, b0:b0 + nrows] = \
                        xhT[ch * P:(ch + 1) * P, row0:row0 + nrows]
                    buf[:, b0 + nrows:b0 + 2 * nrows] = \
                        xlT[ch * P:(ch + 1) * P, row0:row0 + nrows]
        in_maps.append({"xt": buf, "wt": wt, "bias": bias, "biasr": biasr})
    return in_maps


def _gather(results):
    comb = np.concatenate([results[c]["out"] for c in range(N_CORES)], axis=0)
    gate = np.ascontiguousarray(comb[:, :E], dtype=np.float32)
    vals = np.ascontiguousarray(comb[:, E:E + TOPK], dtype=np.float32)
    idx = comb[:, E + TOPK:OW].astype(np.int32)
    return vals, idx, gate


def _run(in_maps, **kw):
    global _prog
    if _prog is None:
        _prog = _build()
    return run_bass_kernel_spmd(_prog, in_maps, list(range(N_CORES)), **kw)


def kernel(x, weight, bias):
    in_maps = _prepare_in_maps(x, weight, bias)
    return _gather(_run(in_maps).results)
